# revision 35
# baseline (speedup 1.0000x reference)
"""Causal self-attention (B=2, T=2048, C=1024, H=16) on 8 TRN2 NeuronCores.

Sharding: core c -> batch b = c//4, heads 4*(c%4) .. 4*(c%4)+3.
Each core computes q,k,v for its 4 heads (column-parallel qkv), causal
attention, and a partial output projection over its heads' rows of
w_proj (row-parallel). Host sums the 4 partials per batch and adds
b_proj.

Device algorithm (per core, SPMD):
  - qT,kT in [cols, tokens] bf16 so attention scores are built
    transposed: S^T[j,i] = k_j . q_i, keys j on partitions, queries i
    on the free dim. Head pairs live at partition offsets 0/64.
  - exp(S^T) on ACT (no max subtraction: |S| <= ~2 by construction),
    written as bf16; causal mask via triu multiply (DVE, bf16 2x) on
    the 128x128 diagonal blocks only; matmul N-ranges restricted below
    the diagonal.
  - P@V accumulated as out^T[d,i] with V (natural [tokens, dh] bf16)
    as the stationary operand; a ones-column appended to V yields the
    softmax denominator l[i] as PSUM row 64 for free.
  - normalization: reciprocal_approx_fast on the PSUM l row ->
    gpsimd partition_broadcast -> DVE multiply into bf16 attT.
  - projection consumes att^T directly as the stationary operand
    against bf16 w_proj.
  - the phase-2 instruction stream is software-pipelined: scores(t+1)
    is emitted before PV(t) so the PE never waits on exp; qkv matmuls
    for the second token half, late v tiles, and projections of
    finished i-blocks are emitted as fillers inside the exp-bound
    diagonal stretches of later blocks.
"""
import numpy as np
import ml_dtypes

import concourse.bacc as bacc
import concourse.bass as bass
import concourse.mybir as mybir
import concourse.tile as tile
from concourse.bass_utils import run_bass_kernel_spmd

F32 = mybir.dt.float32
F32R = mybir.dt.float32r
BF16 = mybir.dt.bfloat16
AF = mybir.ActivationFunctionType

B, T, C = 2, 2048, 1024
H, DH = 16, 64
HPC = 4                    # heads per core
QKCOLS = 2 * HPC * DH      # 512 (q block 256 | k block 256)
VCOLS = HPC * DH           # 256
KC = C // 128              # 8 contraction chunks
TT = T // 128              # 16 token tiles
NB = T // 512              # 4 i-blocks
USE_PSUM_MASK = False      # causal mask via PE accumulate vs DVE triu mul


def build_nc():
    nc = bacc.Bacc("TRN2", target_bir_lowering=False, debug=False, num_devices=8)

    # x and weights are pre-arranged on the host to [p, kc, ...] so every
    # DMA has multi-KB contiguous runs per partition (strided (kc p)->p kc
    # rearranges on the fly cost ~5x in DMA descriptor overhead)
    xT_d = nc.dram_tensor("xT", (128, 4 * KC * 512), BF16, kind="ExternalInput")
    wqk_d = nc.dram_tensor("wqk", (128, KC * QKCOLS), BF16, kind="ExternalInput")
    bqk_d = nc.dram_tensor("bqk", (128, QKCOLS // 128), F32, kind="ExternalInput")
    wv_d = nc.dram_tensor("wv", (128, KC * VCOLS), BF16, kind="ExternalInput")
    bvb_d = nc.dram_tensor("bvb", (128, HPC, DH), BF16, kind="ExternalInput")
    wp_d = nc.dram_tensor("wp", (128, 2 * C), BF16, kind="ExternalInput")
    ident_d = nc.dram_tensor("ident", (128, 128), BF16, kind="ExternalInput")
    mneg_d = nc.dram_tensor("mneg", (128, 128), BF16, kind="ExternalInput")
    triu_d = nc.dram_tensor("triu", (128, 128), BF16, kind="ExternalInput")
    out_d = nc.dram_tensor("out", (T, C), F32, kind="ExternalOutput")

    with tile.TileContext(nc) as tc:
        with (
            tc.tile_pool(name="persist", bufs=1) as pp,
            tc.tile_pool(name="xw", bufs=1) as px,
            tc.tile_pool(name="work", bufs=3) as pw,
            tc.tile_pool(name="psA", bufs=2, space="PSUM") as psA,
            tc.tile_pool(name="psB", bufs=4, space="PSUM") as psB,
        ):
            # ---- constants & weights (persistent) ----
            ident = pp.tile([128, 128], BF16, tag="ident")
            mneg = pp.tile([128, 128], BF16, tag="mneg")
            triu = pp.tile([128, 128], BF16, tag="triu")
            bqk_sb = pp.tile([128, QKCOLS // 128], F32, tag="bqk")
            bvb_sb = pp.tile([128, HPC, DH], BF16, tag="bvb")
            wp_sb = pp.tile([128, 2, C], BF16, tag="wp")

            # persistent activations
            qkT = pp.tile([128, 4, T], BF16, tag="qkT")        # ct: q g0|q g1|k g0|k g1
            v_sb = pp.tile([128, TT, HPC, DH + 1], BF16, tag="v_sb")
            attT = pp.tile([128, 2, T], BF16, tag="attT")

            xT_sb = px.tile([128, KC, T], BF16, tag="xT")
            wqk_sb = px.tile([128, KC, QKCOLS], BF16, tag="wqk")
            wv_sb = px.tile([128, KC, VCOLS], BF16, tag="wv")

            # input DMAs split across rings: x quarters on the sync HWDGE
            # ring, weights on the scalar (ACT) HWDGE ring, consts on the
            # gpsimd SWDGE ring — they stream in parallel.
            nc.gpsimd.dma_start(bqk_sb[:], bqk_d.ap())
            nc.gpsimd.dma_start(ident[:], ident_d.ap())
            nc.gpsimd.dma_start(mneg[:], mneg_d.ap())
            nc.gpsimd.dma_start(triu[:], triu_d.ap())
            nc.gpsimd.dma_start(bvb_sb[:], bvb_d.ap())
            nc.scalar.dma_start(wv_sb[:], wv_d.ap().rearrange("p (kc m) -> p kc m", kc=KC))
            nc.scalar.dma_start(wqk_sb[:], wqk_d.ap().rearrange("p (kc m) -> p kc m", kc=KC))
            nc.scalar.dma_start(wp_sb[:], wp_d.ap().rearrange("p (kc n) -> p kc n", kc=2))
            for q in range(4):
                nc.sync.dma_start(
                    xT_sb[:, :, q * 512 : (q + 1) * 512],
                    xT_d.ap()[:, q * 4096 : (q + 1) * 4096].rearrange(
                        "p (kc t) -> p kc t", kc=KC
                    ),
                )

            # ones column of v (denominator trick), written once
            nc.vector.memset(v_sb[:, :, :, DH : DH + 1], 1.0)

            # initialize the psA pool buffers once so diag-tile exps may
            # read the never-written gap [512:512+so) (sim-visible only)
            for ii in range(2):
                zt = psA.tile([128, 1024], F32, tag="psA", name=f"z{ii}")
                nc.vector.memset(zt[:], 0.0)

            # ---- emission helpers ----
            def emit_v(tt):
                pv = psA.tile([128, 1024], F32, tag="psA", name=f"pv{tt}")
                for kc in range(KC):
                    nc.tensor.matmul(
                        pv[:, 0:VCOLS],
                        xT_sb[:, kc, tt * 128 : (tt + 1) * 128],
                        wv_sb[:, kc, :],
                        start=(kc == 0),
                        stop=(kc == KC - 1),
                    )
                nc.vector.tensor_add(
                    v_sb[:, tt, :, 0:DH],
                    pv[:, 0:VCOLS].rearrange("p (h d) -> p h d", h=HPC),
                    bvb_sb[:],
                )

            def emit_qk(ct, tq):
                pq = psA.tile([128, 1024], F32, tag="psA", name=f"pq{ct}_{tq}")
                lo = tq * 512
                for kc in range(KC):
                    nc.tensor.matmul(
                        pq[:, 0:512],
                        wqk_sb[:, kc, ct * 128 : (ct + 1) * 128],
                        xT_sb[:, kc, lo : lo + 512],
                        start=(kc == 0),
                        stop=(kc == KC - 1),
                    )
                nc.vector.tensor_scalar_add(
                    qkT[:, ct, lo : lo + 512],
                    pq[:, 0:512],
                    bqk_sb[:, ct : ct + 1],
                )

            def emit_proj_tt(tt):
                ps_o = psA.tile([128, 1024], F32, tag="psA", name=f"po{tt}")
                for kc2 in range(2):
                    for half in range(2):
                        nc.tensor.matmul(
                            ps_o[:, half * 512 : (half + 1) * 512],
                            attT[:, kc2, tt * 128 : (tt + 1) * 128],
                            wp_sb[:, kc2, half * 512 : (half + 1) * 512],
                            start=(kc2 == 0),
                            stop=(kc2 == 1),
                        )
                o_sb = pw.tile([128, 1024], F32, tag="osb")
                nc.vector.tensor_copy(o_sb[:], ps_o[:])
                nc.sync.dma_start(out_d.ap()[tt * 128 : (tt + 1) * 128, :], o_sb[:])

            def att_block(bi, g, fillers=None):
                fillers = fillers or {}
                ioff = bi * 512
                njt = 4 * bi + 4
                qT = [qkT[0:64, g, :], qkT[64:128, g, :]]
                kT = [qkT[0:64, 2 + g, :], qkT[64:128, 2 + g, :]]
                oa = [
                    psB.tile([DH + 1, 512], F32, tag="oa", name=f"oa{bi}_{g}_{u}")
                    for u in range(2)
                ]
                prev = None
                for jt in range(njt):
                    d = jt - 4 * bi
                    so = d * 128 if d >= 0 else 0
                    ps_s = psA.tile([128, 1024], F32, tag="psA", name=f"ps{bi}_{g}_{jt}")
                    exp_s = pw.tile([128, 1024], BF16, tag="exp")
                    for u in range(2):
                        nc.tensor.matmul(
                            ps_s[:, u * 512 + so : (u + 1) * 512],
                            kT[u][:, jt * 128 : (jt + 1) * 128],
                            qT[u][:, ioff + so : ioff + 512],
                            start=True,
                            stop=(d < 0 or not USE_PSUM_MASK),
                        )
                        if d >= 0 and USE_PSUM_MASK:
                            # causal mask: accumulate -100 onto the strictly
                            # upper part of the 128x128 diagonal block
                            nc.tensor.matmul(
                                ps_s[:, u * 512 + so : u * 512 + so + 128],
                                ident[:],
                                mneg[:],
                                start=False,
                                stop=True,
                            )
                    # one ACT per tile over [so:1024] — ACT has ~320ns fixed
                    # cost, so fewer/bigger beats per-u splits; the never
                    # written gap [512:512+so] exps garbage nobody reads
                    nc.scalar.activation(
                        exp_s[:, so:1024], ps_s[:, so:1024], AF.Exp
                    )
                    if d >= 0 and not USE_PSUM_MASK:
                        for u in range(2):
                            nc.vector.tensor_mul(
                                exp_s[:, u * 512 + so : u * 512 + so + 128],
                                exp_s[:, u * 512 + so : u * 512 + so + 128],
                                triu[:],
                            )
                    for f in fillers.get(jt, []):
                        f()
                    if prev is not None:
                        pjt, pexp, pso = prev
                        for u in range(2):
                            nc.tensor.matmul(
                                oa[u][:, pso:512],
                                v_sb[:, pjt, 2 * g + u, :],
                                pexp[:, u * 512 + pso : (u + 1) * 512],
                                start=(pjt == 0),
                                stop=(pjt == njt - 1),
                            )
                    prev = (jt, exp_s, so)
                pjt, pexp, pso = prev
                for u in range(2):
                    nc.tensor.matmul(
                        oa[u][:, pso:512],
                        v_sb[:, pjt, 2 * g + u, :],
                        pexp[:, u * 512 + pso : (u + 1) * 512],
                        start=(pjt == 0),
                        stop=True,
                    )
                # normalization for the head pair (reciprocal_approx_fast
                # reads garbage from PSUM on HW — stage l through SBUF)
                for u in range(2):
                    lrow = pw.tile([1, 512], F32, tag="lrow", name=f"lr{bi}_{g}_{u}")
                    nc.vector.tensor_copy(lrow[:], oa[u][DH : DH + 1, :])
                    rst = pw.tile([1, 512], F32, tag="rst", name=f"rst{bi}_{g}_{u}")
                    nc.vector.reciprocal_approx_fast(rst[:], lrow[:])
                    rb = pw.tile([DH, 512], F32, tag="rb")
                    nc.gpsimd.partition_broadcast(rb[:], rst[:])
                    nc.vector.tensor_mul(
                        attT[64 * u : 64 * u + 64, g, ioff : ioff + 512],
                        oa[u][0:DH, :],
                        rb[:],
                    )

            # ---- schedule ----
            # qk col-tiles (ct): 0 = q g0, 1 = q g1, 2 = k g0, 3 = k g1;
            # tq = 512-token quarter. Fine-grained fillers (~0.85-1.7us of
            # PE work) sit every 1-2 tiles to cover the per-tile scalar
            # deficit (exp ~1.25us/tile vs PE ~0.85us/tile) — the PE pays
            # a ~160ns restart penalty whenever it idles, so keeping it
            # 100% fed matters twice over.
            def qk(ct, tq):
                return lambda: emit_qk(ct, tq)

            def v(tt):
                return lambda: emit_v(tt)

            def pj(tt):
                return lambda: emit_proj_tt(tt)

            for tt in range(4):
                emit_v(tt)
            emit_qk(2, 0)
            emit_qk(0, 0)
            att_block(0, 0, {1: [v(4)], 2: [qk(3, 0)], 3: [qk(1, 0)]})
            att_block(0, 1, {1: [v(5)], 2: [qk(0, 1)], 3: [v(6)]})
            att_block(1, 0, {2: [qk(2, 1)], 4: [qk(1, 1)], 6: [v(7)]})
            att_block(1, 1, {2: [qk(3, 1)], 4: [qk(0, 2)], 6: [pj(0)], 7: [pj(1)]})
            att_block(
                2, 0,
                {2: [qk(1, 2)], 4: [qk(2, 2)], 6: [v(8)], 7: [v(9)],
                 8: [v(10)], 10: [v(11)]},
            )
            att_block(
                2, 1,
                {2: [qk(3, 2)], 4: [pj(2)], 6: [qk(0, 3)], 8: [pj(3)],
                 10: [pj(4)]},
            )
            att_block(
                3, 0,
                {2: [qk(1, 3)], 4: [v(12)], 6: [v(13)], 7: [qk(2, 3)],
                 9: [v(14)], 11: [v(15)], 13: [pj(5)]},
            )
            att_block(
                3, 1,
                {2: [qk(3, 3)], 4: [pj(6)], 6: [pj(7)], 8: [pj(8)],
                 10: [pj(9)], 12: [pj(10)], 14: [pj(11)]},
            )
            for tt in range(12, 16):
                emit_proj_tt(tt)

    nc.compile()
    return nc


def make_core_inputs(x, w_qkv, b_qkv, w_proj, b_proj):
    """Per-core input maps (host-side sharding)."""
    x = np.asarray(x, dtype=np.float32)
    w_qkv = np.asarray(w_qkv, dtype=np.float32)
    b_qkv = np.asarray(b_qkv, dtype=np.float32)
    w_proj = np.asarray(w_proj, dtype=np.float32)

    ident = np.eye(128, dtype=ml_dtypes.bfloat16)
    # S^T layout [keys j, queries i]: mask j > i, i.e. the strict lower part
    mneg = np.tril(
        np.full((128, 128), -100.0, dtype=np.float32), k=-1
    ).astype(ml_dtypes.bfloat16)
    triu = np.triu(np.ones((128, 128), dtype=np.float32)).astype(ml_dtypes.bfloat16)
    in_maps = []
    for c in range(8):
        b = c // 4
        heads = [4 * (c % 4) + i for i in range(HPC)]
        qcols = np.concatenate([np.arange(64 * h, 64 * h + 64) for h in heads])
        wq = w_qkv[:, qcols] * 0.125
        bq = b_qkv[qcols] * 0.125
        wk = w_qkv[:, C + qcols]
        bk = b_qkv[C + qcols]
        wv = w_qkv[:, 2 * C + qcols]
        bv = b_qkv[2 * C + qcols]
        wqk = np.ascontiguousarray(np.concatenate([wq, wk], axis=1))
        bqk = np.concatenate([bq, bk]).reshape(QKCOLS // 128, 128).T.copy()
        bvb = np.broadcast_to(
            bv.reshape(1, HPC, DH), (128, HPC, DH)
        ).astype(ml_dtypes.bfloat16)
        wp = np.ascontiguousarray(w_proj[qcols, :]).astype(ml_dtypes.bfloat16)
        # pre-arrange to [p, ...] layouts with contiguous per-partition runs
        xT = x[b].T.astype(ml_dtypes.bfloat16)              # [C, T]
        xP = np.ascontiguousarray(
            xT.reshape(KC, 128, 4, 512).transpose(1, 2, 0, 3).reshape(128, 4 * KC * 512)
        )
        wqkP = np.ascontiguousarray(
            wqk.astype(ml_dtypes.bfloat16).reshape(KC, 128, QKCOLS)
            .transpose(1, 0, 2).reshape(128, KC * QKCOLS)
        )
        wvP = np.ascontiguousarray(
            wv.astype(ml_dtypes.bfloat16).reshape(KC, 128, VCOLS)
            .transpose(1, 0, 2).reshape(128, KC * VCOLS)
        )
        wpP = np.ascontiguousarray(
            wp.reshape(2, 128, C).transpose(1, 0, 2).reshape(128, 2 * C)
        )
        in_maps.append({
            "xT": xP,
            "wqk": wqkP,
            "bqk": bqk,
            "wv": wvP,
            "bvb": np.ascontiguousarray(bvb),
            "wp": wpP,
            "ident": ident,
            "mneg": mneg,
            "triu": triu,
        })
    return in_maps


_NC_CACHE = []


def kernel(x, w_qkv, b_qkv, w_proj, b_proj):
    if not _NC_CACHE:
        _NC_CACHE.append(build_nc())
    nc = _NC_CACHE[0]
    in_maps = make_core_inputs(x, w_qkv, b_qkv, w_proj, b_proj)
    res = run_bass_kernel_spmd(nc, in_maps, list(range(8)))
    b_proj = np.asarray(b_proj, dtype=np.float32)
    out = np.empty((B, T, C), dtype=np.float32)
    for b in range(B):
        acc = res.results[4 * b]["out"].astype(np.float32).copy()
        for c in range(4 * b + 1, 4 * b + 4):
            acc += res.results[c]["out"]
        out[b] = acc + b_proj
    return out
